# revision 23
# baseline (speedup 1.0000x reference)
"""Trainium2 Bass kernel for nn_BertGTHead_37177236914708 (BertGT pooling head).

Full-input contract: kernel(**inputs) takes the complete (unsharded) numpy
inputs and returns the full [B, 1+G] float32 output.

Strategy (data-parallel over batch, 2 examples per NeuronCore, 8 cores):
  - All pooling ops (text max/avg, window max/avg) only touch tokens with
    base_mask=1 (~50% of the sequence for this distribution). The host
    compacts each example's valid rows into a dense, order-preserving fp16
    array (zero-padded to a build-time capacity C); the device then streams
    ~C instead of S rows per example with NO mask multiplies at all:
    zero-padding is exact for the sum and absorbed by the final max(.,0).
  - text pooling: stream the compacted array in [128, GRP*H] fp16 chunks,
    ex0 on the sync HWDGE ring and ex1 on the scalar ring (behind the tiny
    aux loads) so both examples' chunks complete early and in order;
    running elementwise max on VectorE; sums on the PE with a ones column
    as the fp16 stationary operand, accumulated in PSUM; partition-axis
    max finalized via PE transposes + free-axis reduce from SBUF.
  - window pooling: order-preserving compaction makes each gap's valid
    window a contiguous range [a_g, b_g) of the compacted array. Blocks of
    32 rows around a_g are fetched by ONE indirect DMA (issued as early as
    possible) as [(ob,ex,g) partitions x 8 rows]; slot masks applied on
    ScalarE; sum/max trees on VectorE; cross-block fold via PE transposes,
    an ACT PSUM->SBUF copy and three cheap TT ops. The whole window
    section is emitted mid-stream so it fills engine gaps instead of
    serializing after the stream.
  - centers (gap rows, unmasked) are host-gathered and uploaded densely.
  - final scores: combined per-partition dot (center|max|avg vs relaid-out
    weights), reduced on VectorE, then a single ones-matmul on the PE sums
    the 128 h-partials for all 34 outputs at once, in output order.

Everything index/mask-shaped is precomputed on the host; all O(B*S*H)
reductions run on the NeuronCores in fp16/fp32.
"""

import numpy as np
from contextlib import ExitStack

# ---- problem constants (hardcoded; harness runs kernel.py standalone) ----
B, S, H, G = 16, 4096, 768, 16
WIN = 15
WLEN = 2 * WIN + 1           # 31
NCORES = 8
EX = B // NCORES             # 2 examples per core
P = 128
GRP = 2                      # token rows per partition per stream chunk
CHUNK = P * GRP              # 256 compacted rows per stream chunk
C_MIN = 2304                 # default capacity (valid rows per example)
OB = 4                       # 8-row blocks per (32-row padded) window
OB_R = 8                     # rows per block
NOUT = 1 + G                 # 17 scores per example

_BUILT = {}                  # capacity C -> compiled Bacc
_C = C_MIN


def _build(C):
    """Build + compile the per-core Bass program for capacity C (cached)."""
    if C in _BUILT:
        return _BUILT[C]

    import concourse.bacc as bacc
    import concourse.bass as bass
    import concourse.tile as tile
    from concourse import mybir
    from concourse.masks import make_identity

    f16 = mybir.dt.float16
    f32 = mybir.dt.float32
    i32 = mybir.dt.int32
    AF = mybir.ActivationFunctionType
    OP = mybir.AluOpType
    AX = mybir.AxisListType

    NCH = C // CHUNK         # stream chunks per example
    NE = EX * G              # 32

    nc = bacc.Bacc("TRN2", target_bir_lowering=False, debug=False,
                   num_devices=NCORES)

    xc_d = nc.dram_tensor("xc", [EX * C, H], f16, kind="ExternalInput").ap()
    ctr_d = nc.dram_tensor("ctr", [NE, H], f32, kind="ExternalInput").ap()
    auxf_d = nc.dram_tensor("auxf", [P, 86], f32, kind="ExternalInput").ap()
    cw3row_d = nc.dram_tensor("cw3row", [1, EX * H], f32, kind="ExternalInput").ap()
    winidx_d = nc.dram_tensor("winidx", [P, 1], i32, kind="ExternalInput").ap()
    wmask_d = nc.dram_tensor("wmask", [P, OB_R], f32, kind="ExternalInput").ap()
    out_d = nc.dram_tensor("out", [EX * NOUT], f32, kind="ExternalOutput").ap()

    with tile.TileContext(nc) as tc, ExitStack() as ctx:
        singles = ctx.enter_context(tc.tile_pool(name="singles", bufs=1))
        xpool = ctx.enter_context(tc.tile_pool(name="xin", bufs=EX * NCH))
        accpool = ctx.enter_context(tc.tile_pool(name="acc", bufs=2))
        winpool = ctx.enter_context(tc.tile_pool(name="win", bufs=1))
        smalls = ctx.enter_context(tc.tile_pool(name="smalls", bufs=4))
        foldp = ctx.enter_context(tc.tile_pool(name="fold", bufs=2))
        pacc = ctx.enter_context(tc.tile_pool(name="pacc", bufs=2, space="PSUM"))
        pbig = ctx.enter_context(tc.tile_pool(name="pbig", bufs=1, space="PSUM"))
        pbigc = ctx.enter_context(tc.tile_pool(name="pbigc", bufs=1, space="PSUM"))
        pout = ctx.enter_context(tc.tile_pool(name="pout", bufs=1, space="PSUM"))

        # ---- all small aux loads ride the GpSimd SWDGE queue so the two
        # HWDGE rings carry nothing but stream chunks (HWDGE descriptor
        # generation, ~25ns/descriptor, is what paces the stream) ----
        winidx_sb = singles.tile([P, 1], i32)
        nc.gpsimd.dma_start(out=winidx_sb[:], in_=winidx_d)

        # both examples' T=1 chunks ride the fast SWDGE generation path so
        # the two running-max chains can start ~6us earlier
        x3 = bass.AP(xc_d.tensor, 0, [[GRP * H, EX * C // GRP], [1, GRP * H]])
        xts = {}
        for ex in range(EX):
            xt = xpool.tile([P, GRP * H], f16, tag="xt")
            row0 = ex * (C // GRP) + P
            nc.gpsimd.dma_start(out=xt[:], in_=x3[row0: row0 + P, :])
            xts[(ex, 1)] = xt

        # window gather: one indirect DMA (winidx data has landed by now)
        xrow = bass.AP(xc_d.tensor, 0, [[H, EX * C], [1, H]])
        wt = winpool.tile([P, OB_R * H], f16)
        nc.gpsimd.indirect_dma_start(
            out=wt[:], out_offset=None, in_=xrow,
            in_offset=bass.IndirectOffsetOnAxis(ap=winidx_sb[:], axis=0))

        wmask_sb = singles.tile([P, OB_R], f32)
        nc.gpsimd.dma_start(out=wmask_sb[:], in_=wmask_d)
        auxf_sb = singles.tile([P, 86], f32)
        nc.gpsimd.dma_start(out=auxf_sb[:], in_=auxf_d)
        cw3row_sb = singles.tile([1, EX * H], f32)
        nc.gpsimd.dma_start(out=cw3row_sb[:], in_=cw3row_d)
        ct = winpool.tile([NE, H], f32)
        nc.gpsimd.dma_start(out=ct[:], in_=ctr_d)

        # ---- remaining stream chunk DMAs: ex0 owns the sync ring, ex1
        # the scalar ring, nothing ahead of them ----
        for T in range(NCH):
            if T == 1:
                continue
            for ex in range(EX):
                xt = xpool.tile([P, GRP * H], f16, tag="xt")
                row0 = ex * (C // GRP) + T * P
                eng = nc.sync if ex == 0 else nc.scalar
                eng.dma_start(out=xt[:], in_=x3[row0: row0 + P, :])
                xts[(ex, T)] = xt

        ident = singles.tile([P, P], f32)
        make_identity(nc, ident[:])
        ones = singles.tile([P, 1], f32)
        nc.vector.memset(ones[:], 1.0)
        ones16 = singles.tile([P, 1], f16)
        nc.vector.memset(ones16[:], 1.0)
        # one-time ACT table load, after the scalar-ring DMA issues
        warm = singles.tile([1, 1], f32)
        nc.scalar.activation(out=warm[:], in_=ones[0:1, 0:1], func=AF.Copy)

        pooled_a = auxf_sb[:, 0:12]
        cwc_a = auxf_sb[:, 12:36]
        invcnt_a = auxf_sb[:, 36:68]
        gwt_a = auxf_sb[:, 68:86]

        # rhs of the final ones-matmul, in output order:
        # col ex*17 = cls partials, cols ex*17+1+g = gap partials
        rhs34 = smalls.tile([P, EX * NOUT], f32)

        # ---- stream compute pieces ----
        accs = [accpool.tile([P, GRP * H], f16, name=f"acc{e}", tag="acc")
                for e in range(EX)]
        pss = [pacc.tile([1, H], f32, name=f"ps{e}", tag="ps")
               for e in range(EX)]

        def stream_T(T):
            for ex in range(EX):
                xt = xts[(ex, T)]
                ps = pss[ex]
                for j in range(GRP):
                    first = (T == 0 and j == 0)
                    last = (T == NCH - 1 and j == GRP - 1)
                    nc.tensor.matmul(out=ps[0:1, 0:512],
                                     lhsT=ones16[:],
                                     rhs=xt[:, j * H:j * H + 512],
                                     start=first, stop=last)
                    nc.tensor.matmul(out=ps[0:1, 512:H],
                                     lhsT=ones16[:],
                                     rhs=xt[:, j * H + 512:(j + 1) * H],
                                     start=first, stop=last)
                if T == 1:
                    nc.vector.tensor_tensor(out=accs[ex][:],
                                            in0=xts[(ex, 0)][:],
                                            in1=xt[:], op=OP.max)
                elif T > 1:
                    nc.vector.tensor_tensor(out=accs[ex][:], in0=accs[ex][:],
                                            in1=xt[:], op=OP.max)

        ws = winpool.tile([P, OB_R * H // 2], f16)
        wsF = winpool.tile([P, H], f32)
        wm = winpool.tile([P, OB_R * H // 2], f16)
        wtF = winpool.tile([P, H], f32)
        gfeat = winpool.tile([P, 3 * 6 * NE], f32)       # [cT|maxT|sumT]
        gfold = winpool.tile([P, 2 * 6 * NE], f32)

        def emit_window_a():
            # slot masks on ScalarE (per-partition scale, one op per slot)
            for o in range(OB_R):
                nc.scalar.activation(out=wt[:, o * H:(o + 1) * H],
                                     in_=wt[:, o * H:(o + 1) * H],
                                     func=AF.Copy, scale=wmask_sb[:, o:o + 1])
            # sum tree on VectorE (non-destructive; wt stays for the max)
            nc.vector.tensor_tensor(out=ws[:], in0=wt[:, 0:4 * H],
                                    in1=wt[:, 4 * H:8 * H], op=OP.add)
            nc.vector.tensor_tensor(out=ws[:, 0:2 * H], in0=ws[:, 0:2 * H],
                                    in1=ws[:, 2 * H:4 * H], op=OP.add)
            nc.vector.tensor_tensor(out=wsF[:], in0=ws[:, 0:H],
                                    in1=ws[:, H:2 * H], op=OP.add)
            # max tree on VectorE (own scratch)
            nc.vector.tensor_tensor(out=wm[:], in0=wt[:, 0:4 * H],
                                    in1=wt[:, 4 * H:8 * H], op=OP.max)
            nc.vector.tensor_tensor(out=wm[:, 0:2 * H], in0=wm[:, 0:2 * H],
                                    in1=wm[:, 2 * H:4 * H], op=OP.max)
            nc.vector.tensor_tensor(out=wtF[:], in0=wm[:, 0:H],
                                    in1=wm[:, H:2 * H], op=OP.max)
            # centers can be transposed as soon as ct + ident are loaded
            ptC = pbigc.tile([P, 6 * NE], f32)
            for c in range(6):
                nc.tensor.transpose(out=ptC[:, c * NE:(c + 1) * NE],
                                    in_=ct[:, c * P:(c + 1) * P],
                                    identity=ident[0:NE, 0:NE])
            nc.scalar.activation(out=gfeat[:, 0:6 * NE], in_=ptC[:],
                                 func=AF.Copy)

        def emit_window_b():
            # transpose to h-partition layout, copy PSUM->SBUF on ScalarE,
            # fold the 4 ob-groups with cheap TT ops on GpSimd (idle here,
            # runs parallel to the stream finalization on VectorE)
            def obfold(gm, dst, op):
                # gm free layout: c*128 + ob*32 + e (c in 6, ob in 4, e in 32)
                g = gm[:]
                v = [bass.AP(g.tensor, g.offset + ob * NE,
                             [g.ap[0], [P, 6], [1, NE]]) for ob in range(OB)]
                f = gfold[:]
                f01 = bass.AP(f.tensor, f.offset, [f.ap[0], [NE, 6], [1, NE]])
                f23 = bass.AP(f.tensor, f.offset + 6 * NE,
                              [f.ap[0], [NE, 6], [1, NE]])
                d = bass.AP(dst.tensor, dst.offset,
                            [dst.ap[0], [NE, 6], [1, NE]])
                nc.vector.tensor_tensor(out=f01, in0=v[0], in1=v[1], op=op)
                nc.vector.tensor_tensor(out=f23, in0=v[2], in1=v[3], op=op)
                nc.vector.tensor_tensor(out=d, in0=f01, in1=f23, op=op)

            ptM = pbig.tile([P, H], f32, tag="ptw")
            for c in range(6):
                nc.tensor.transpose(out=ptM[:, c * P:(c + 1) * P],
                                    in_=wtF[:, c * P:(c + 1) * P],
                                    identity=ident[:])
            gmM = winpool.tile([P, H], f32)
            nc.scalar.activation(out=gmM[:], in_=ptM[:], func=AF.Copy)
            obfold(gmM, gfeat[:, 6 * NE:12 * NE], OP.max)
            nc.vector.tensor_scalar_max(out=gfeat[:, 6 * NE:12 * NE],
                                        in0=gfeat[:, 6 * NE:12 * NE],
                                        scalar1=0.0)
            ptS = pbig.tile([P, H], f32, tag="ptw")
            for c in range(6):
                nc.tensor.transpose(out=ptS[:, c * P:(c + 1) * P],
                                    in_=wsF[:, c * P:(c + 1) * P],
                                    identity=ident[:])
            gmS = winpool.tile([P, H], f32)
            nc.scalar.activation(out=gmS[:], in_=ptS[:], func=AF.Copy)
            obfold(gmS, gfeat[:, 12 * NE:18 * NE], OP.add)
            # avg = sum / cnt  (per (ex,g) along free)
            icnt_b = bass.AP(invcnt_a.tensor, invcnt_a.offset,
                             [invcnt_a.ap[0], [0, 6], [1, NE]])
            gf_s = bass.AP(gfeat[:].tensor, gfeat[:].offset + 12 * NE,
                           [gfeat[:].ap[0], [NE, 6], [1, NE]])
            nc.vector.tensor_tensor(out=gf_s, in0=gf_s, in1=icnt_b,
                                    op=OP.mult)

            # combined gap dot: gfeat[p,(part,c,exg)] * W[part*H + c*128 + p]
            gw_b = bass.AP(gwt_a.tensor, gwt_a.offset,
                           [gwt_a.ap[0], [6, 3], [1, 6], [0, NE]])
            gf_v = bass.AP(gfeat[:].tensor, gfeat[:].offset,
                           [gfeat[:].ap[0], [6 * NE, 3], [NE, 6], [1, NE]])
            nc.vector.tensor_tensor(out=gf_v, in0=gf_v, in1=gw_b, op=OP.mult)
            gf_r = bass.AP(gfeat[:].tensor, gfeat[:].offset,
                           [gfeat[:].ap[0], [1, NE], [NE, 18]])
            rhs_g = bass.AP(rhs34[:].tensor, rhs34[:].offset + 1,
                            [rhs34[:].ap[0], [NOUT, EX], [1, G]])
            nc.vector.tensor_reduce(out=rhs_g, in_=gf_r, axis=AX.X,
                                    op=OP.add)

        # ---- emission order: early stream chunks, window mults+trees
        # (ready mid-stream), remaining chunks, per-example finalization,
        # then the window fold/dot tail ----
        for T in range(min(2, NCH)):
            stream_T(T)
        emit_window_a()
        for T in range(2, NCH):
            stream_T(T)

        # ---- stream finalization per example ----
        for ex in range(EX):
            acc = accs[ex]
            ps = pss[ex]
            maxf = foldp.tile([P, H], f32)
            nc.vector.tensor_tensor(out=maxf[:], in0=acc[:, 0:H],
                                    in1=acc[:, H:2 * H], op=OP.max)
            pt = pbig.tile([P, H], f32, tag="ptw")
            for c in range(6):
                nc.tensor.transpose(out=pt[:, c * P:(c + 1) * P],
                                    in_=maxf[:, c * P:(c + 1) * P],
                                    identity=ident[:])
            ptsb = foldp.tile([P, H], f32)
            nc.scalar.activation(out=ptsb[:], in_=pt[:], func=AF.Copy)
            feat6 = foldp.tile([P, 6], f32)
            pt_v = ptsb[:].rearrange("p (c s) -> p c s", c=6)
            nc.vector.tensor_reduce(out=feat6[:], in_=pt_v, axis=AX.X,
                                    op=OP.max)
            # zero-padding may be absent (nv == C): floor at 0 here
            nc.vector.tensor_scalar_max(out=feat6[:], in0=feat6[:],
                                        scalar1=0.0)

            # cls partials from pooled & text-max features (h-partitioned)
            cprod = foldp.tile([P, 12], f32)
            nc.vector.tensor_tensor(out=cprod[:, 0:6],
                                    in0=pooled_a[:, ex * 6:(ex + 1) * 6],
                                    in1=cwc_a[:, ex * 12:ex * 12 + 6],
                                    op=OP.mult)
            nc.vector.tensor_tensor(out=cprod[:, 6:12], in0=feat6[:],
                                    in1=cwc_a[:, ex * 12 + 6:ex * 12 + 12],
                                    op=OP.mult)
            cidx = ex * NOUT
            nc.vector.tensor_reduce(out=rhs34[:, cidx:cidx + 1],
                                    in_=cprod[:], axis=AX.X, op=OP.add)

            # text-sum (avg) contribution: ps . cw3row (single partition,
            # on GpSimd so it overlaps the VectorE tail)
            cprod3 = foldp.tile([1, H], f32)
            nc.vector.tensor_tensor(out=cprod3[:], in0=ps[:],
                                    in1=cw3row_sb[0:1, ex * H:(ex + 1) * H],
                                    op=OP.mult)
            red3 = foldp.tile([1, 1], f32)
            nc.vector.tensor_reduce(out=red3[:], in_=cprod3[:], axis=AX.X,
                                    op=OP.add)
            nc.vector.tensor_tensor(out=rhs34[0:1, cidx:cidx + 1],
                                    in0=rhs34[0:1, cidx:cidx + 1],
                                    in1=red3[0:1, 0:1], op=OP.add)

        emit_window_b()

        # ---- final ones-matmul (sums partials over h' partitions) ----
        pscore = pout.tile([1, EX * NOUT], f32)
        nc.tensor.matmul(out=pscore[:], lhsT=ones[:], rhs=rhs34[:],
                         start=True, stop=True)
        sg = smalls.tile([1, EX * NOUT], f32)
        nc.scalar.activation(out=sg[:], in_=pscore[:], func=AF.Copy)
        nc.sync.dma_start(out=out_d[:], in_=sg[0:1, :])

    nc.compile()
    _BUILT[C] = nc
    return nc


def _prep_core(seq_c, pooled_c, bm_c, gids_c, gW, cW, C):
    """Host-side per-core input prep. seq_c [EX,S,H] f32 (view), bm_c [EX,S]
    bool, gids_c [EX,G] int, gW [3H] f32, cW [3H] f32, C = capacity."""
    f32 = np.float32
    f16 = np.float16

    xc = np.zeros((EX * C, H), dtype=f16)
    a = np.empty((EX, G), dtype=np.int64)
    b = np.empty((EX, G), dtype=np.int64)
    tn = np.empty((EX,), dtype=f32)
    for ex in range(EX):
        pos = np.flatnonzero(bm_c[ex])
        nv = len(pos)
        tn[ex] = nv
        xc[ex * C:ex * C + nv] = seq_c[ex, pos].astype(f16)
        a[ex] = np.searchsorted(pos, gids_c[ex] - WIN, side="left")
        b[ex] = np.searchsorted(pos, gids_c[ex] + WIN, side="right")

    # window partitions: p = ob*32 + ex*16 + g; each reads OB_R=8 compacted
    # rows starting at row start + ob*8 of a 32-row padded block
    NE = EX * G
    obv = np.repeat(np.arange(OB), NE)            # [P]
    exv = np.tile(np.repeat(np.arange(EX), G), OB)
    gv = np.tile(np.arange(G), EX * OB)
    a_p = a[exv, gv]                              # [P]
    b_p = b[exv, gv]
    start = np.clip(a_p, 0, C - OB * OB_R)        # [P] padded-block start
    winidx = (exv * C + start + obv * OB_R).astype(np.int32).reshape(P, 1)
    rows = (start + obv * OB_R)[:, None] + np.arange(OB_R)[None, :]  # [P, 8]
    wmask = ((rows >= a_p[:, None]) & (rows < b_p[:, None])).astype(f32)

    cnt = (b - a).astype(f32)                     # [EX, G]
    with np.errstate(divide="ignore"):
        icnt = 1.0 / cnt

    exg_e = np.repeat(np.arange(EX), G)
    exg_g = np.tile(np.arange(G), EX)
    ctr = np.ascontiguousarray(
        seq_c[exg_e, gids_c[exg_e, exg_g]], dtype=f32)     # [NE, H]

    # auxf[:, 0:12] pooledr, [:, 12:36] cwc, [:, 36:68] invcnt, [:, 68:86] gwt
    auxf = np.empty((P, 86), f32)
    cw12 = cW[:2 * H].reshape(2, 6, P)            # [part, c, p]
    cw3row = np.empty((1, EX * H), f32)
    for ex in range(EX):
        auxf[:, ex * 6:(ex + 1) * 6] = pooled_c[ex].reshape(6, P).T
        auxf[:, 12 + ex * 12:12 + ex * 12 + 6] = cw12[0].T
        auxf[:, 12 + ex * 12 + 6:12 + ex * 12 + 12] = cw12[1].T
        cw3row[0, ex * H:(ex + 1) * H] = cW[2 * H:] / tn[ex]
    auxf[:, 36:68] = np.broadcast_to(icnt.reshape(NE), (P, NE))
    auxf[:, 68:86] = gW.reshape(3, 6, P).transpose(2, 0, 1).reshape(P, 18)

    return {
        "xc": xc,
        "ctr": ctr,
        "auxf": auxf,
        "cw3row": cw3row,
        "winidx": winidx,
        "wmask": wmask,
    }


def _make_in_maps(sequence_output, pooled_output, token_type_ids, word_mask,
                  gap_ids, gap_W, cls_W):
    global _C
    seq = np.asarray(sequence_output, dtype=np.float32)
    pooled = np.asarray(pooled_output, dtype=np.float32)
    tti = np.asarray(token_type_ids)
    wmk = np.asarray(word_mask)
    gids = np.asarray(gap_ids).astype(np.int64)
    gW = np.asarray(gap_W, dtype=np.float32)
    cW = np.asarray(cls_W, dtype=np.float32)
    base_mask = (tti == 0) & (wmk != 0)

    max_nv = int(base_mask.sum(axis=1).max())
    C = max(C_MIN, -(-max_nv // CHUNK) * CHUNK)
    # keep the compiled capacity if it still fits (avoid rebuilds)
    if _BUILT and any(c >= C for c in _BUILT):
        C = min(c for c in _BUILT if c >= C)
    _C = C

    in_maps = []
    for c in range(NCORES):
        lo = c * EX
        in_maps.append(_prep_core(seq[lo:lo + EX], pooled[lo:lo + EX],
                                  base_mask[lo:lo + EX], gids[lo:lo + EX],
                                  gW, cW, C))
    return in_maps


def _run(in_maps, trace=False, trace_cores=None):
    from concourse import bass_utils
    nc = _build(_C)
    return bass_utils.run_bass_kernel_spmd(
        nc, in_maps, core_ids=list(range(NCORES)), trace=trace,
        trace_cores=trace_cores)


def kernel(sequence_output, pooled_output, token_type_ids, word_mask,
           gap_ids, gap_W, gap_b, cls_W, cls_b):
    in_maps = _make_in_maps(sequence_output, pooled_output, token_type_ids,
                            word_mask, gap_ids, gap_W, cls_W)
    res = _run(in_maps)
    out = np.concatenate(
        [res.results[c]["out"].reshape(EX, NOUT) for c in range(NCORES)], 0)
    out[:, 0] += float(np.asarray(cls_b))
    out[:, 1:] += float(np.asarray(gap_b))
    return out.astype(np.float32)


# revision 24
# speedup vs baseline: 1.0620x; 1.0620x over previous
"""Trainium2 Bass kernel for nn_BertGTHead_37177236914708 (BertGT pooling head).

Full-input contract: kernel(**inputs) takes the complete (unsharded) numpy
inputs and returns the full [B, 1+G] float32 output.

Strategy (data-parallel over batch, 2 examples per NeuronCore, 8 cores):
  - All pooling ops (text max/avg, window max/avg) only touch tokens with
    base_mask=1 (~50% of the sequence for this distribution). The host
    compacts each example's valid rows into a dense, order-preserving fp16
    array (zero-padded to a build-time capacity C); the device then streams
    ~C instead of S rows per example with NO mask multiplies at all:
    zero-padding is exact for the sum and absorbed by the final max(.,0).
  - text pooling: stream the compacted array in [128, GRP*H] fp16 chunks,
    ex0 on the sync HWDGE ring and ex1 on the scalar ring (behind the tiny
    aux loads) so both examples' chunks complete early and in order;
    running elementwise max on VectorE; sums on the PE with a ones column
    as the fp16 stationary operand, accumulated in PSUM; partition-axis
    max finalized via PE transposes + free-axis reduce from SBUF.
  - window pooling: order-preserving compaction makes each gap's valid
    window a contiguous range [a_g, b_g) of the compacted array. Blocks of
    32 rows around a_g are fetched by ONE indirect DMA (issued as early as
    possible) as [(ob,ex,g) partitions x 8 rows]; slot masks applied on
    ScalarE; sum/max trees on VectorE; cross-block fold via PE transposes,
    an ACT PSUM->SBUF copy and three cheap TT ops. The whole window
    section is emitted mid-stream so it fills engine gaps instead of
    serializing after the stream.
  - centers (gap rows, unmasked) are host-gathered and uploaded densely.
  - final scores: combined per-partition dot (center|max|avg vs relaid-out
    weights), reduced on VectorE, then a single ones-matmul on the PE sums
    the 128 h-partials for all 34 outputs at once, in output order.

Everything index/mask-shaped is precomputed on the host; all O(B*S*H)
reductions run on the NeuronCores in fp16/fp32.
"""

import numpy as np
from contextlib import ExitStack

# ---- problem constants (hardcoded; harness runs kernel.py standalone) ----
B, S, H, G = 16, 4096, 768, 16
WIN = 15
WLEN = 2 * WIN + 1           # 31
NCORES = 8
EX = B // NCORES             # 2 examples per core
P = 128
GRP = 2                      # token rows per partition per stream chunk
CHUNK = P * GRP              # 256 compacted rows per stream chunk
C_MIN = 2304                 # default capacity (valid rows per example)
OB = 4                       # 8-row blocks per (32-row padded) window
OB_R = 8                     # rows per block
NOUT = 1 + G                 # 17 scores per example

_BUILT = {}                  # capacity C -> compiled Bacc
_C = C_MIN


def _build(C):
    """Build + compile the per-core Bass program for capacity C (cached)."""
    if C in _BUILT:
        return _BUILT[C]

    import concourse.bacc as bacc
    import concourse.bass as bass
    import concourse.tile as tile
    from concourse import mybir
    from concourse.masks import make_identity

    f16 = mybir.dt.float16
    f32 = mybir.dt.float32
    i32 = mybir.dt.int32
    AF = mybir.ActivationFunctionType
    OP = mybir.AluOpType
    AX = mybir.AxisListType

    NCH = C // CHUNK         # stream chunks per example
    NE = EX * G              # 32

    nc = bacc.Bacc("TRN2", target_bir_lowering=False, debug=False,
                   num_devices=NCORES)

    xc_d = nc.dram_tensor("xc", [EX * C, H], f16, kind="ExternalInput").ap()
    ctr_d = nc.dram_tensor("ctr", [NE, H], f32, kind="ExternalInput").ap()
    auxf_d = nc.dram_tensor("auxf", [P, 86], f32, kind="ExternalInput").ap()
    cw3row_d = nc.dram_tensor("cw3row", [1, EX * H], f32, kind="ExternalInput").ap()
    winidx_d = nc.dram_tensor("winidx", [P, 1], i32, kind="ExternalInput").ap()
    wmask_d = nc.dram_tensor("wmask", [P, OB_R], f32, kind="ExternalInput").ap()
    out_d = nc.dram_tensor("out", [EX * NOUT], f32, kind="ExternalOutput").ap()

    with tile.TileContext(nc) as tc, ExitStack() as ctx:
        singles = ctx.enter_context(tc.tile_pool(name="singles", bufs=1))
        xpool = ctx.enter_context(tc.tile_pool(name="xin", bufs=EX * NCH))
        accpool = ctx.enter_context(tc.tile_pool(name="acc", bufs=2))
        winpool = ctx.enter_context(tc.tile_pool(name="win", bufs=1))
        smalls = ctx.enter_context(tc.tile_pool(name="smalls", bufs=4))
        foldp = ctx.enter_context(tc.tile_pool(name="fold", bufs=2))
        pacc = ctx.enter_context(tc.tile_pool(name="pacc", bufs=2, space="PSUM"))
        pbig = ctx.enter_context(tc.tile_pool(name="pbig", bufs=1, space="PSUM"))
        pbigc = ctx.enter_context(tc.tile_pool(name="pbigc", bufs=1, space="PSUM"))
        pout = ctx.enter_context(tc.tile_pool(name="pout", bufs=1, space="PSUM"))

        # ---- all small aux loads ride the GpSimd SWDGE queue so the two
        # HWDGE rings carry nothing but stream chunks (HWDGE descriptor
        # generation, ~25ns/descriptor, is what paces the stream) ----
        winidx_sb = singles.tile([P, 1], i32)
        nc.gpsimd.dma_start(out=winidx_sb[:], in_=winidx_d)

        # window gather: one indirect DMA, issued as early as possible
        xrow = bass.AP(xc_d.tensor, 0, [[H, EX * C], [1, H]])
        wt = winpool.tile([P, OB_R * H], f16)
        nc.gpsimd.indirect_dma_start(
            out=wt[:], out_offset=None, in_=xrow,
            in_offset=bass.IndirectOffsetOnAxis(ap=winidx_sb[:], axis=0))

        wmask_sb = singles.tile([P, OB_R], f32)
        nc.gpsimd.dma_start(out=wmask_sb[:], in_=wmask_d)
        auxf_sb = singles.tile([P, 86], f32)
        nc.gpsimd.dma_start(out=auxf_sb[:], in_=auxf_d)
        cw3row_sb = singles.tile([1, EX * H], f32)
        nc.gpsimd.dma_start(out=cw3row_sb[:], in_=cw3row_d)
        ct = winpool.tile([NE, H], f32)
        nc.gpsimd.dma_start(out=ct[:], in_=ctr_d)

        # ---- stream chunk DMAs: ex0 owns the sync ring, ex1 the scalar
        # ring, nothing ahead of them ----
        x3 = bass.AP(xc_d.tensor, 0, [[GRP * H, EX * C // GRP], [1, GRP * H]])
        xts = {}
        for T in range(NCH):
            for ex in range(EX):
                xt = xpool.tile([P, GRP * H], f16, tag="xt")
                row0 = ex * (C // GRP) + T * P
                eng = nc.sync if ex == 0 else nc.scalar
                eng.dma_start(out=xt[:], in_=x3[row0: row0 + P, :])
                xts[(ex, T)] = xt

        ident = singles.tile([P, P], f32)
        make_identity(nc, ident[:])
        ones = singles.tile([P, 1], f32)
        nc.vector.memset(ones[:], 1.0)
        ones16 = singles.tile([P, 1], f16)
        nc.vector.memset(ones16[:], 1.0)
        # one-time ACT table load, after the scalar-ring DMA issues
        warm = singles.tile([1, 1], f32)
        nc.scalar.activation(out=warm[:], in_=ones[0:1, 0:1], func=AF.Copy)

        pooled_a = auxf_sb[:, 0:12]
        cwc_a = auxf_sb[:, 12:36]
        invcnt_a = auxf_sb[:, 36:68]
        gwt_a = auxf_sb[:, 68:86]

        # rhs of the final ones-matmul, in output order:
        # col ex*17 = cls partials, cols ex*17+1+g = gap partials
        rhs34 = smalls.tile([P, EX * NOUT], f32)

        # ---- stream compute pieces ----
        accs = [accpool.tile([P, GRP * H], f16, name=f"acc{e}", tag="acc")
                for e in range(EX)]
        pss = [pacc.tile([1, H], f32, name=f"ps{e}", tag="ps")
               for e in range(EX)]

        def stream_T(T):
            for ex in range(EX):
                xt = xts[(ex, T)]
                ps = pss[ex]
                for j in range(GRP):
                    first = (T == 0 and j == 0)
                    last = (T == NCH - 1 and j == GRP - 1)
                    nc.tensor.matmul(out=ps[0:1, 0:512],
                                     lhsT=ones16[:],
                                     rhs=xt[:, j * H:j * H + 512],
                                     start=first, stop=last)
                    nc.tensor.matmul(out=ps[0:1, 512:H],
                                     lhsT=ones16[:],
                                     rhs=xt[:, j * H + 512:(j + 1) * H],
                                     start=first, stop=last)
                if T == 1:
                    nc.vector.tensor_tensor(out=accs[ex][:],
                                            in0=xts[(ex, 0)][:],
                                            in1=xt[:], op=OP.max)
                elif T > 1:
                    nc.vector.tensor_tensor(out=accs[ex][:], in0=accs[ex][:],
                                            in1=xt[:], op=OP.max)

        ws = winpool.tile([P, OB_R * H // 2], f16)
        wsF = winpool.tile([P, H], f32)
        wm = winpool.tile([P, OB_R * H // 2], f16)
        wtF = winpool.tile([P, H], f32)
        gfeat = winpool.tile([P, 3 * 6 * NE], f32)       # [cT|maxT|sumT]
        gfold = winpool.tile([P, 2 * 6 * NE], f32)

        def emit_window_a():
            # slot masks on ScalarE (per-partition scale, one op per slot)
            for o in range(OB_R):
                nc.scalar.activation(out=wt[:, o * H:(o + 1) * H],
                                     in_=wt[:, o * H:(o + 1) * H],
                                     func=AF.Copy, scale=wmask_sb[:, o:o + 1])
            # sum tree on VectorE (non-destructive; wt stays for the max)
            nc.vector.tensor_tensor(out=ws[:], in0=wt[:, 0:4 * H],
                                    in1=wt[:, 4 * H:8 * H], op=OP.add)
            nc.vector.tensor_tensor(out=ws[:, 0:2 * H], in0=ws[:, 0:2 * H],
                                    in1=ws[:, 2 * H:4 * H], op=OP.add)
            nc.vector.tensor_tensor(out=wsF[:], in0=ws[:, 0:H],
                                    in1=ws[:, H:2 * H], op=OP.add)
            # max tree on VectorE (own scratch)
            nc.vector.tensor_tensor(out=wm[:], in0=wt[:, 0:4 * H],
                                    in1=wt[:, 4 * H:8 * H], op=OP.max)
            nc.vector.tensor_tensor(out=wm[:, 0:2 * H], in0=wm[:, 0:2 * H],
                                    in1=wm[:, 2 * H:4 * H], op=OP.max)
            nc.vector.tensor_tensor(out=wtF[:], in0=wm[:, 0:H],
                                    in1=wm[:, H:2 * H], op=OP.max)
            # centers can be transposed as soon as ct + ident are loaded
            ptC = pbigc.tile([P, 6 * NE], f32)
            for c in range(6):
                nc.tensor.transpose(out=ptC[:, c * NE:(c + 1) * NE],
                                    in_=ct[:, c * P:(c + 1) * P],
                                    identity=ident[0:NE, 0:NE])
            nc.scalar.activation(out=gfeat[:, 0:6 * NE], in_=ptC[:],
                                 func=AF.Copy)

        def emit_window_b():
            # transpose to h-partition layout, copy PSUM->SBUF on ScalarE,
            # fold the 4 ob-groups with cheap TT ops on GpSimd (idle here,
            # runs parallel to the stream finalization on VectorE)
            def obfold(gm, dst, op):
                # gm free layout: c*128 + ob*32 + e (c in 6, ob in 4, e in 32)
                g = gm[:]
                v = [bass.AP(g.tensor, g.offset + ob * NE,
                             [g.ap[0], [P, 6], [1, NE]]) for ob in range(OB)]
                f = gfold[:]
                f01 = bass.AP(f.tensor, f.offset, [f.ap[0], [NE, 6], [1, NE]])
                f23 = bass.AP(f.tensor, f.offset + 6 * NE,
                              [f.ap[0], [NE, 6], [1, NE]])
                d = bass.AP(dst.tensor, dst.offset,
                            [dst.ap[0], [NE, 6], [1, NE]])
                nc.vector.tensor_tensor(out=f01, in0=v[0], in1=v[1], op=op)
                nc.vector.tensor_tensor(out=f23, in0=v[2], in1=v[3], op=op)
                nc.vector.tensor_tensor(out=d, in0=f01, in1=f23, op=op)

            ptM = pbig.tile([P, H], f32, tag="ptw")
            for c in range(6):
                nc.tensor.transpose(out=ptM[:, c * P:(c + 1) * P],
                                    in_=wtF[:, c * P:(c + 1) * P],
                                    identity=ident[:])
            gmM = winpool.tile([P, H], f32)
            nc.scalar.activation(out=gmM[:], in_=ptM[:], func=AF.Copy)
            obfold(gmM, gfeat[:, 6 * NE:12 * NE], OP.max)
            nc.vector.tensor_scalar_max(out=gfeat[:, 6 * NE:12 * NE],
                                        in0=gfeat[:, 6 * NE:12 * NE],
                                        scalar1=0.0)
            ptS = pbig.tile([P, H], f32, tag="ptw")
            for c in range(6):
                nc.tensor.transpose(out=ptS[:, c * P:(c + 1) * P],
                                    in_=wsF[:, c * P:(c + 1) * P],
                                    identity=ident[:])
            gmS = winpool.tile([P, H], f32)
            nc.scalar.activation(out=gmS[:], in_=ptS[:], func=AF.Copy)
            obfold(gmS, gfeat[:, 12 * NE:18 * NE], OP.add)
            # avg = sum / cnt  (per (ex,g) along free)
            icnt_b = bass.AP(invcnt_a.tensor, invcnt_a.offset,
                             [invcnt_a.ap[0], [0, 6], [1, NE]])
            gf_s = bass.AP(gfeat[:].tensor, gfeat[:].offset + 12 * NE,
                           [gfeat[:].ap[0], [NE, 6], [1, NE]])
            nc.vector.tensor_tensor(out=gf_s, in0=gf_s, in1=icnt_b,
                                    op=OP.mult)

            # combined gap dot: gfeat[p,(part,c,exg)] * W[part*H + c*128 + p]
            gw_b = bass.AP(gwt_a.tensor, gwt_a.offset,
                           [gwt_a.ap[0], [6, 3], [1, 6], [0, NE]])
            gf_v = bass.AP(gfeat[:].tensor, gfeat[:].offset,
                           [gfeat[:].ap[0], [6 * NE, 3], [NE, 6], [1, NE]])
            nc.vector.tensor_tensor(out=gf_v, in0=gf_v, in1=gw_b, op=OP.mult)
            gf_r = bass.AP(gfeat[:].tensor, gfeat[:].offset,
                           [gfeat[:].ap[0], [1, NE], [NE, 18]])
            rhs_g = bass.AP(rhs34[:].tensor, rhs34[:].offset + 1,
                            [rhs34[:].ap[0], [NOUT, EX], [1, G]])
            nc.vector.tensor_reduce(out=rhs_g, in_=gf_r, axis=AX.X,
                                    op=OP.add)

        # ---- emission order: early stream chunks, window mults+trees
        # (ready mid-stream), remaining chunks, per-example finalization,
        # then the window fold/dot tail ----
        for T in range(min(2, NCH)):
            stream_T(T)
        emit_window_a()
        for T in range(2, NCH):
            stream_T(T)

        # ---- stream finalization per example ----
        for ex in range(EX):
            acc = accs[ex]
            ps = pss[ex]
            maxf = foldp.tile([P, H], f32)
            nc.vector.tensor_tensor(out=maxf[:], in0=acc[:, 0:H],
                                    in1=acc[:, H:2 * H], op=OP.max)
            pt = pbig.tile([P, H], f32, tag="ptw")
            for c in range(6):
                nc.tensor.transpose(out=pt[:, c * P:(c + 1) * P],
                                    in_=maxf[:, c * P:(c + 1) * P],
                                    identity=ident[:])
            ptsb = foldp.tile([P, H], f32)
            nc.scalar.activation(out=ptsb[:], in_=pt[:], func=AF.Copy)
            feat6 = foldp.tile([P, 6], f32)
            pt_v = ptsb[:].rearrange("p (c s) -> p c s", c=6)
            nc.vector.tensor_reduce(out=feat6[:], in_=pt_v, axis=AX.X,
                                    op=OP.max)
            # zero-padding may be absent (nv == C): floor at 0 here
            nc.vector.tensor_scalar_max(out=feat6[:], in0=feat6[:],
                                        scalar1=0.0)

            # cls partials from pooled & text-max features (h-partitioned)
            cprod = foldp.tile([P, 12], f32)
            nc.vector.tensor_tensor(out=cprod[:, 0:6],
                                    in0=pooled_a[:, ex * 6:(ex + 1) * 6],
                                    in1=cwc_a[:, ex * 12:ex * 12 + 6],
                                    op=OP.mult)
            nc.vector.tensor_tensor(out=cprod[:, 6:12], in0=feat6[:],
                                    in1=cwc_a[:, ex * 12 + 6:ex * 12 + 12],
                                    op=OP.mult)
            cidx = ex * NOUT
            nc.vector.tensor_reduce(out=rhs34[:, cidx:cidx + 1],
                                    in_=cprod[:], axis=AX.X, op=OP.add)

            # text-sum (avg) contribution: ps . cw3row (single partition,
            # on GpSimd so it overlaps the VectorE tail)
            cprod3 = foldp.tile([1, H], f32)
            nc.vector.tensor_tensor(out=cprod3[:], in0=ps[:],
                                    in1=cw3row_sb[0:1, ex * H:(ex + 1) * H],
                                    op=OP.mult)
            red3 = foldp.tile([1, 1], f32)
            nc.vector.tensor_reduce(out=red3[:], in_=cprod3[:], axis=AX.X,
                                    op=OP.add)
            nc.vector.tensor_tensor(out=rhs34[0:1, cidx:cidx + 1],
                                    in0=rhs34[0:1, cidx:cidx + 1],
                                    in1=red3[0:1, 0:1], op=OP.add)

        emit_window_b()

        # ---- final ones-matmul (sums partials over h' partitions) ----
        pscore = pout.tile([1, EX * NOUT], f32)
        nc.tensor.matmul(out=pscore[:], lhsT=ones[:], rhs=rhs34[:],
                         start=True, stop=True)
        sg = smalls.tile([1, EX * NOUT], f32)
        nc.scalar.activation(out=sg[:], in_=pscore[:], func=AF.Copy)
        nc.sync.dma_start(out=out_d[:], in_=sg[0:1, :])

    nc.compile()
    _BUILT[C] = nc
    return nc


def _prep_core(seq_c, pooled_c, bm_c, gids_c, gW, cW, C):
    """Host-side per-core input prep. seq_c [EX,S,H] f32 (view), bm_c [EX,S]
    bool, gids_c [EX,G] int, gW [3H] f32, cW [3H] f32, C = capacity."""
    f32 = np.float32
    f16 = np.float16

    xc = np.zeros((EX * C, H), dtype=f16)
    a = np.empty((EX, G), dtype=np.int64)
    b = np.empty((EX, G), dtype=np.int64)
    tn = np.empty((EX,), dtype=f32)
    for ex in range(EX):
        pos = np.flatnonzero(bm_c[ex])
        nv = len(pos)
        tn[ex] = nv
        xc[ex * C:ex * C + nv] = seq_c[ex, pos].astype(f16)
        a[ex] = np.searchsorted(pos, gids_c[ex] - WIN, side="left")
        b[ex] = np.searchsorted(pos, gids_c[ex] + WIN, side="right")

    # window partitions: p = ob*32 + ex*16 + g; each reads OB_R=8 compacted
    # rows starting at row start + ob*8 of a 32-row padded block
    NE = EX * G
    obv = np.repeat(np.arange(OB), NE)            # [P]
    exv = np.tile(np.repeat(np.arange(EX), G), OB)
    gv = np.tile(np.arange(G), EX * OB)
    a_p = a[exv, gv]                              # [P]
    b_p = b[exv, gv]
    start = np.clip(a_p, 0, C - OB * OB_R)        # [P] padded-block start
    winidx = (exv * C + start + obv * OB_R).astype(np.int32).reshape(P, 1)
    rows = (start + obv * OB_R)[:, None] + np.arange(OB_R)[None, :]  # [P, 8]
    wmask = ((rows >= a_p[:, None]) & (rows < b_p[:, None])).astype(f32)

    cnt = (b - a).astype(f32)                     # [EX, G]
    with np.errstate(divide="ignore"):
        icnt = 1.0 / cnt

    exg_e = np.repeat(np.arange(EX), G)
    exg_g = np.tile(np.arange(G), EX)
    ctr = np.ascontiguousarray(
        seq_c[exg_e, gids_c[exg_e, exg_g]], dtype=f32)     # [NE, H]

    # auxf[:, 0:12] pooledr, [:, 12:36] cwc, [:, 36:68] invcnt, [:, 68:86] gwt
    auxf = np.empty((P, 86), f32)
    cw12 = cW[:2 * H].reshape(2, 6, P)            # [part, c, p]
    cw3row = np.empty((1, EX * H), f32)
    for ex in range(EX):
        auxf[:, ex * 6:(ex + 1) * 6] = pooled_c[ex].reshape(6, P).T
        auxf[:, 12 + ex * 12:12 + ex * 12 + 6] = cw12[0].T
        auxf[:, 12 + ex * 12 + 6:12 + ex * 12 + 12] = cw12[1].T
        cw3row[0, ex * H:(ex + 1) * H] = cW[2 * H:] / tn[ex]
    auxf[:, 36:68] = np.broadcast_to(icnt.reshape(NE), (P, NE))
    auxf[:, 68:86] = gW.reshape(3, 6, P).transpose(2, 0, 1).reshape(P, 18)

    return {
        "xc": xc,
        "ctr": ctr,
        "auxf": auxf,
        "cw3row": cw3row,
        "winidx": winidx,
        "wmask": wmask,
    }


def _make_in_maps(sequence_output, pooled_output, token_type_ids, word_mask,
                  gap_ids, gap_W, cls_W):
    global _C
    seq = np.asarray(sequence_output, dtype=np.float32)
    pooled = np.asarray(pooled_output, dtype=np.float32)
    tti = np.asarray(token_type_ids)
    wmk = np.asarray(word_mask)
    gids = np.asarray(gap_ids).astype(np.int64)
    gW = np.asarray(gap_W, dtype=np.float32)
    cW = np.asarray(cls_W, dtype=np.float32)
    base_mask = (tti == 0) & (wmk != 0)

    max_nv = int(base_mask.sum(axis=1).max())
    C = max(C_MIN, -(-max_nv // CHUNK) * CHUNK)
    # keep the compiled capacity if it still fits (avoid rebuilds)
    if _BUILT and any(c >= C for c in _BUILT):
        C = min(c for c in _BUILT if c >= C)
    _C = C

    in_maps = []
    for c in range(NCORES):
        lo = c * EX
        in_maps.append(_prep_core(seq[lo:lo + EX], pooled[lo:lo + EX],
                                  base_mask[lo:lo + EX], gids[lo:lo + EX],
                                  gW, cW, C))
    return in_maps


def _run(in_maps, trace=False, trace_cores=None):
    from concourse import bass_utils
    nc = _build(_C)
    return bass_utils.run_bass_kernel_spmd(
        nc, in_maps, core_ids=list(range(NCORES)), trace=trace,
        trace_cores=trace_cores)


def kernel(sequence_output, pooled_output, token_type_ids, word_mask,
           gap_ids, gap_W, gap_b, cls_W, cls_b):
    in_maps = _make_in_maps(sequence_output, pooled_output, token_type_ids,
                            word_mask, gap_ids, gap_W, cls_W)
    res = _run(in_maps)
    out = np.concatenate(
        [res.results[c]["out"].reshape(EX, NOUT) for c in range(NCORES)], 0)
    out[:, 0] += float(np.asarray(cls_b))
    out[:, 1:] += float(np.asarray(gap_b))
    return out.astype(np.float32)
